# revision 10
# baseline (speedup 1.0000x reference)
"""Trainium2 Bass kernel for nn_Attention (B=4,T=2048,C=512,H=8 causal RoPE attention).

Sharding: 8 cores = 4 batches x 2 head-groups. Core c handles batch c//2 and
heads [4*(c%2), 4*(c%2)+4). Each core computes its proj partial y_part[T, C];
the host sums the two partials per batch and adds bp.

Per-core dataflow (all channels-on-partitions "transposed" layouts):
  qT = Wq_loc @ x^T, plus a pair-swapped copy qTs = Wq_swap @ x^T.
  RoPE: qT_rot = qT*cos + qTs*sin_signed   (elementwise, host-built tables)
  Scores are computed transposed: S^T[kt, qt] = k_rot^T.T-slices @ q_rot^T,
  exp via ACT (scale=1/8 folded in, no max subtraction - scores are O(0.5)),
  causal masking via 0/1 mask multiply on the 4 diagonal tiles per q-chunk.
  PV: out^T[d, qt] accumulates (V|1)^T-stationary matmuls; row 64 = softmax
  denominator. Normalize via ones-column K=1 matmul broadcast + multiply,
  then y_part = out_norm^T.T @ Wp_loc^T chunks.
"""

import sys

for _p in ("/opt/trn_rl_repo",):
    if _p not in sys.path:
        sys.path.insert(0, _p)

from contextlib import ExitStack

import ml_dtypes
import numpy as np

import concourse.bass as bass
import concourse.tile as tile
from concourse import bacc
from concourse import mybir
from concourse.bass_utils import run_bass_kernel_spmd


def _ensure_ntff_hook():
    """Provide antenv.axon_hooks (missing in this image) so trace=True works."""
    try:
        import antenv.axon_hooks  # noqa: F401

        return
    except ImportError:
        pass
    import contextlib
    import ctypes
    import types

    import antenv

    mod = types.ModuleType("antenv.axon_hooks")
    holder = {}
    mod.set_axon_ntff_profile_hook = lambda h: holder.__setitem__("h", h)
    mod.get_axon_ntff_profile_hook = lambda: holder.get("h")
    antenv.axon_hooks = mod
    sys.modules["antenv.axon_hooks"] = mod

    so_path = "/opt/axon/libaxon_pjrt.so"
    try:
        lib = ctypes.CDLL(so_path)
    except OSError:
        return
    if not hasattr(lib, "axon_start_nrt_profile"):
        return
    lib.axon_start_nrt_profile.argtypes = [
        ctypes.POINTER(ctypes.c_int64),
        ctypes.c_size_t,
    ]
    lib.axon_start_nrt_profile.restype = ctypes.c_int64
    lib.axon_stop_nrt_profile.argtypes = [ctypes.c_char_p]
    lib.axon_stop_nrt_profile.restype = ctypes.c_int64

    @contextlib.contextmanager
    def _hook(output_dir, device_ids):
        import jax

        jax.devices()
        if device_ids:
            ids = (ctypes.c_int64 * len(device_ids))(*device_ids)
            rc = lib.axon_start_nrt_profile(ids, len(device_ids))
        else:
            rc = lib.axon_start_nrt_profile(None, 0)
        if rc != 0:
            raise RuntimeError(f"axon_start_nrt_profile rc={rc}")
        try:
            yield
        finally:
            n = lib.axon_stop_nrt_profile(str(output_dir).encode())
            print(f"profile: {n} file(s) written to {output_dir}", file=sys.stderr)

    mod.set_axon_ntff_profile_hook(_hook)

BF16 = mybir.dt.bfloat16
F32 = mybir.dt.float32
NPBF = ml_dtypes.bfloat16

B, C, H, D = 4, 512, 8, 64
HPC = 4              # heads per core
CL = HPC * D         # 256 local channels
NCORES = 8
THETA = 10000.0
QC = 512             # q-chunk width (free dim per matmul)
KT = 128             # kt tile (partition dim of S^T tiles)
GROUP = 3            # kt-tiles per exp group (3 PSUM banks)
ACT_EXP = mybir.ActivationFunctionType.Exp


def build_nc(T: int) -> bass.Bass:
    PT = T // 128
    NJ = T // QC
    nc = bacc.Bacc()

    xT = nc.declare_dram_parameter("xT", [C, T], BF16, isOutput=False)
    wq = nc.declare_dram_parameter("wq", [C, CL], BF16, isOutput=False)
    wqs = nc.declare_dram_parameter("wqs", [C, CL], BF16, isOutput=False)
    wk = nc.declare_dram_parameter("wk", [C, CL], BF16, isOutput=False)
    wks = nc.declare_dram_parameter("wks", [C, CL], BF16, isOutput=False)
    wv = nc.declare_dram_parameter("wv", [C, CL], BF16, isOutput=False)
    wp = nc.declare_dram_parameter("wp", [CL, C], BF16, isOutput=False)
    cosb = nc.declare_dram_parameter("cosb", [CL, T], F32, isOutput=False)
    sinb = nc.declare_dram_parameter("sinb", [CL, T], F32, isOutput=False)
    msk = nc.declare_dram_parameter("msk", [4 * 128, QC], BF16, isOutput=False)
    y = nc.declare_dram_parameter("y", [T, C], F32, isOutput=True)

    with nc.allow_low_precision(
        reason="bf16 compute by design; f32 PSUM accumulation everywhere"
    ), tile.TileContext(nc) as tc, ExitStack() as ctx:
        pers = ctx.enter_context(tc.tile_pool(name="pers", bufs=1))
        work = ctx.enter_context(tc.tile_pool(name="work", bufs=4))
        pexp = ctx.enter_context(tc.tile_pool(name="pexp", bufs=3))
        big = ctx.enter_context(tc.tile_pool(name="big", bufs=2, space="PSUM"))
        b1 = ctx.enter_context(tc.tile_pool(name="b1", bufs=2, space="PSUM"))

        # ---------------- persistent SBUF: inputs ----------------
        xT_sb = [pers.tile([128, T], BF16, name=f"xT{i}", tag=f"xT{i}") for i in range(4)]
        for i in range(4):
            nc.sync.dma_start(out=xT_sb[i][:], in_=xT[128 * i:128 * i + 128, :])

        def load_w(handle, name):
            ts = [pers.tile([128, CL], BF16, name=f"{name}{i}", tag=f"{name}{i}") for i in range(4)]
            for i in range(4):
                nc.sync.dma_start(out=ts[i][:], in_=handle[128 * i:128 * i + 128, :])
            return ts

        wq_sb = load_w(wq, "wq")
        wqs_sb = load_w(wqs, "wqs")
        wk_sb = load_w(wk, "wk")
        wks_sb = load_w(wks, "wks")
        wv_sb = load_w(wv, "wv")
        wp_sb = [pers.tile([128, C], BF16, name=f"wp{i}", tag=f"wp{i}") for i in range(2)]
        for i in range(2):
            nc.sync.dma_start(out=wp_sb[i][:], in_=wp[128 * i:128 * i + 128, :])
        cos_sb = [pers.tile([128, T], F32, name=f"cos{i}", tag=f"cos{i}") for i in range(2)]
        sin_sb = [pers.tile([128, T], F32, name=f"sin{i}", tag=f"sin{i}") for i in range(2)]
        for i in range(2):
            nc.sync.dma_start(out=cos_sb[i][:], in_=cosb[128 * i:128 * i + 128, :])
            nc.sync.dma_start(out=sin_sb[i][:], in_=sinb[128 * i:128 * i + 128, :])
        msk_sb = [pers.tile([128, QC], BF16, name=f"msk{r}", tag=f"msk{r}") for r in range(4)]
        for r in range(4):
            nc.sync.dma_start(out=msk_sb[r][:], in_=msk[128 * r:128 * r + 128, :])

        # ---------------- persistent SBUF: intermediates ----------------
        qT_sb = [pers.tile([128, T], BF16, name=f"qT{i}", tag=f"qT{i}") for i in range(2)]
        kT_sb = [pers.tile([128, T], BF16, name=f"kT{i}", tag=f"kT{i}") for i in range(2)]
        vx_sb = [pers.tile([128, HPC * (D + 1)], BF16, name=f"vx{i}", tag=f"vx{i}") for i in range(PT)]
        rawT_sb = [pers.tile([128, T], F32, name=f"raw{i}", tag=f"raw{i}") for i in range(2)]
        rnT_sb = [pers.tile([128, T], BF16, name=f"rn{i}", tag=f"rn{i}") for i in range(2)]
        den_sb = pers.tile([1, HPC * T], BF16, name="den", tag="den")
        ones_sb = pers.tile([1, 64], BF16, name="ones", tag="ones")
        nc.vector.memset(ones_sb[:], 1.0)

        # ---------------- stage B: q/k projections + RoPE ----------------
        for wn, ws, dst in ((wq_sb, wqs_sb, qT_sb), (wk_sb, wks_sb, kT_sb)):
            for m in range(2):
                for t4 in range(NJ):
                    pq = big.tile([128, GROUP * 512], F32, name="big", tag="big")
                    tsl = slice(QC * t4, QC * t4 + QC)
                    for kc in range(4):
                        nc.tensor.matmul(
                            pq[:, 0:512],
                            lhsT=wn[kc][:, 128 * m:128 * m + 128],
                            rhs=xT_sb[kc][:, tsl],
                            start=(kc == 0),
                            stop=(kc == 3),
                        )
                    for kc in range(4):
                        nc.tensor.matmul(
                            pq[:, 512:1024],
                            lhsT=ws[kc][:, 128 * m:128 * m + 128],
                            rhs=xT_sb[kc][:, tsl],
                            start=(kc == 0),
                            stop=(kc == 3),
                        )
                    t1 = work.tile([128, 512], F32, name="t1", tag="t1")
                    t2 = work.tile([128, 512], F32, name="t2", tag="t2")
                    nc.vector.tensor_mul(t1[:], pq[:, 0:512], cos_sb[m][:, tsl])
                    nc.vector.tensor_mul(t2[:], pq[:, 512:1024], sin_sb[m][:, tsl])
                    nc.vector.tensor_add(dst[m][:, tsl], t1[:], t2[:])

        # ---------------- stage B2: v projection (natural layout + ones col) --
        for tt in range(PT):
            pv = b1.tile([128, 512], F32, name="b1", tag="b1")
            for kc in range(4):
                nc.tensor.matmul(
                    pv[:, 0:CL],
                    lhsT=xT_sb[kc][:, 128 * tt:128 * tt + 128],
                    rhs=wv_sb[kc][:],
                    start=(kc == 0),
                    stop=(kc == 3),
                )
            for h in range(HPC):
                nc.vector.tensor_copy(
                    vx_sb[tt][:, 65 * h:65 * h + 64], pv[:, 64 * h:64 * h + 64]
                )
                nc.vector.memset(vx_sb[tt][:, 65 * h + 64:65 * h + 65], 1.0)

        # ---------------- stage C: attention ----------------
        for h in range(HPC):
            ph, po = h // 2, 64 * (h % 2)
            for j in range(NJ):
                nkt = 4 * (j + 1)
                pvp = b1.tile([128, 512], F32, name="b1", tag="b1")
                qsl = slice(QC * j, QC * j + QC)
                its = list(range(nkt))
                for u0 in range(0, nkt, GROUP):
                    grp = its[u0:u0 + GROUP]
                    sg = big.tile([128, GROUP * 512], F32, name="big", tag="big")
                    for ui, it in enumerate(grp):
                        nc.tensor.matmul(
                            sg[:, 512 * ui:512 * ui + 512],
                            lhsT=kT_sb[ph][po:po + 64, 128 * it:128 * it + 128],
                            rhs=qT_sb[ph][po:po + 64, qsl],
                            start=True,
                            stop=True,
                        )
                    w = 512 * len(grp)
                    pg = pexp.tile([128, GROUP * 512], BF16, name="pg", tag="pg")
                    nc.scalar.activation(pg[:, 0:w], sg[:, 0:w], ACT_EXP, scale=0.125)
                    for ui, it in enumerate(grp):
                        psl = slice(512 * ui, 512 * ui + 512)
                        r = it - 4 * j
                        if r >= 0:
                            nc.gpsimd.tensor_mul(pg[:, psl], pg[:, psl], msk_sb[r][:])
                        nc.tensor.matmul(
                            pvp[0:65, :],
                            lhsT=vx_sb[it][:, 65 * h:65 * h + 65],
                            rhs=pg[:, psl],
                            start=(it == 0),
                            stop=(it == nkt - 1),
                        )
                nc.vector.tensor_copy(rawT_sb[ph][po:po + 64, qsl], pvp[0:64, :])
                nc.vector.tensor_copy(
                    den_sb[0:1, h * T + QC * j:h * T + QC * j + QC], pvp[64:65, :]
                )

        # ---------------- stage N: normalize ----------------
        for ph in range(2):
            for j in range(NJ):
                qsl = slice(QC * j, QC * j + QC)
                bc = b1.tile([128, 512], F32, name="b1", tag="b1")
                for sub in range(2):
                    h = 2 * ph + sub
                    nc.tensor.matmul(
                        bc[64 * sub:64 * sub + 64, :],
                        lhsT=ones_sb[0:1, :],
                        rhs=den_sb[0:1, h * T + QC * j:h * T + QC * j + QC],
                        start=True,
                        stop=True,
                        tile_position=(0, 64 * sub),
                    )
                rbc = work.tile([128, 512], F32, name="rbc", tag="rbc")
                nc.vector.reciprocal_approx_fast(rbc[:], bc[:])
                nc.vector.tensor_mul(
                    rnT_sb[ph][:, qsl], rawT_sb[ph][:, qsl], rbc[:]
                )

        # ---------------- stage D: output projection ----------------
        for tt in range(PT):
            pp = b1.tile([128, 512], F32, name="b1", tag="b1")
            for kc in range(2):
                nc.tensor.matmul(
                    pp[:],
                    lhsT=rnT_sb[kc][:, 128 * tt:128 * tt + 128],
                    rhs=wp_sb[kc][:],
                    start=(kc == 0),
                    stop=(kc == 1),
                )
            ys = work.tile([128, 512], F32, name="ys", tag="ys")
            nc.vector.tensor_copy(ys[:], pp[:])
            nc.sync.dma_start(out=y[128 * tt:128 * tt + 128, :], in_=ys[:])

    nc.finalize()
    return nc


def prep_core_inputs(x, Wq, Wk, Wv, Wp, core, T):
    b, g = core // 2, core % 2
    sl = slice(CL * g, CL * g + CL)
    sw = np.arange(CL) ^ 1
    wq_ = np.ascontiguousarray(Wq[sl, :].T)
    wk_ = np.ascontiguousarray(Wk[sl, :].T)
    lc = np.arange(CL)
    gpair = (CL * g + lc) // 2
    invf = THETA ** (-(2.0 * gpair) / C)
    ang = np.arange(T)[None, :] * invf[:, None]
    cosb = np.cos(ang).astype(np.float32)
    sgn = np.where(lc % 2 == 0, -1.0, 1.0)
    sinb = (np.sin(ang) * sgn[:, None]).astype(np.float32)
    # causal 0/1 masks for the 4 diagonal kt-tiles of each q-chunk
    p = np.arange(128)[:, None]
    q = np.arange(QC)[None, :]
    m = np.concatenate(
        [(q >= 128 * r + p).astype(np.float32) for r in range(4)], axis=0
    )
    return {
        "xT": np.ascontiguousarray(x[b].T).astype(NPBF),
        "wq": wq_.astype(NPBF),
        "wqs": np.ascontiguousarray(wq_[:, sw]).astype(NPBF),
        "wk": wk_.astype(NPBF),
        "wks": np.ascontiguousarray(wk_[:, sw]).astype(NPBF),
        "wv": np.ascontiguousarray(Wv[sl, :].T).astype(NPBF),
        "wp": np.ascontiguousarray(Wp[:, sl].T).astype(NPBF),
        "cosb": cosb,
        "sinb": sinb,
        "msk": m.astype(NPBF),
    }


_NC_CACHE = {}


def _get_nc(T):
    if T not in _NC_CACHE:
        _NC_CACHE[T] = build_nc(T)
    return _NC_CACHE[T]


def kernel(x, Wq, Wk, Wv, Wp, bp, _trace=False):
    x = np.asarray(x, dtype=np.float32)
    Wq = np.asarray(Wq, dtype=np.float32)
    Wk = np.asarray(Wk, dtype=np.float32)
    Wv = np.asarray(Wv, dtype=np.float32)
    Wp = np.asarray(Wp, dtype=np.float32)
    bp = np.asarray(bp, dtype=np.float32)
    T = x.shape[1]
    nc = _get_nc(T)
    in_maps = [prep_core_inputs(x, Wq, Wk, Wv, Wp, c, T) for c in range(NCORES)]
    if _trace:
        _ensure_ntff_hook()
    res = run_bass_kernel_spmd(nc, in_maps, list(range(NCORES)), trace=_trace)
    out = np.zeros((B, T, C), np.float32)
    for b in range(B):
        out[b] = res.results[2 * b]["y"] + res.results[2 * b + 1]["y"]
    out += bp[None, None, :]
    if _trace:
        return out, res
    return out


# revision 11
# speedup vs baseline: 1.3184x; 1.3184x over previous
"""Trainium2 Bass kernel for nn_Attention (B=4,T=2048,C=512,H=8 causal RoPE attention).

Sharding: 8 cores = 4 batches x 2 head-groups. Core c handles batch c//2 and
heads [4*(c%2), 4*(c%2)+4). Each core computes its proj partial y_part[T, C];
the host sums the two partials per batch and adds bp.

Per-core dataflow (all channels-on-partitions "transposed" layouts):
  qT = Wq_loc @ x^T, plus a pair-swapped copy qTs = Wq_swap @ x^T.
  RoPE: qT_rot = qT*cos + qTs*sin_signed   (elementwise, host-built tables)
  Scores are computed transposed: S^T[kt, qt] = k_rot^T.T-slices @ q_rot^T,
  exp via ACT (scale=1/8 folded in, no max subtraction - scores are O(0.5)),
  causal masking via 0/1 mask multiply on the 4 diagonal tiles per q-chunk.
  PV: out^T[d, qt] accumulates (V|1)^T-stationary matmuls; row 64 = softmax
  denominator. Normalize via ones-column K=1 matmul broadcast + multiply,
  then y_part = out_norm^T.T @ Wp_loc^T chunks.
"""

import sys

for _p in ("/opt/trn_rl_repo",):
    if _p not in sys.path:
        sys.path.insert(0, _p)

from contextlib import ExitStack

import ml_dtypes
import numpy as np

import concourse.bass as bass
import concourse.tile as tile
from concourse import bacc
from concourse import mybir
from concourse.bass_utils import run_bass_kernel_spmd


def _ensure_ntff_hook():
    """Provide antenv.axon_hooks (missing in this image) so trace=True works."""
    try:
        import antenv.axon_hooks  # noqa: F401

        return
    except ImportError:
        pass
    import contextlib
    import ctypes
    import types

    import antenv

    mod = types.ModuleType("antenv.axon_hooks")
    holder = {}
    mod.set_axon_ntff_profile_hook = lambda h: holder.__setitem__("h", h)
    mod.get_axon_ntff_profile_hook = lambda: holder.get("h")
    antenv.axon_hooks = mod
    sys.modules["antenv.axon_hooks"] = mod

    so_path = "/opt/axon/libaxon_pjrt.so"
    try:
        lib = ctypes.CDLL(so_path)
    except OSError:
        return
    if not hasattr(lib, "axon_start_nrt_profile"):
        return
    lib.axon_start_nrt_profile.argtypes = [
        ctypes.POINTER(ctypes.c_int64),
        ctypes.c_size_t,
    ]
    lib.axon_start_nrt_profile.restype = ctypes.c_int64
    lib.axon_stop_nrt_profile.argtypes = [ctypes.c_char_p]
    lib.axon_stop_nrt_profile.restype = ctypes.c_int64

    @contextlib.contextmanager
    def _hook(output_dir, device_ids):
        import jax

        jax.devices()
        if device_ids:
            ids = (ctypes.c_int64 * len(device_ids))(*device_ids)
            rc = lib.axon_start_nrt_profile(ids, len(device_ids))
        else:
            rc = lib.axon_start_nrt_profile(None, 0)
        if rc != 0:
            raise RuntimeError(f"axon_start_nrt_profile rc={rc}")
        try:
            yield
        finally:
            n = lib.axon_stop_nrt_profile(str(output_dir).encode())
            print(f"profile: {n} file(s) written to {output_dir}", file=sys.stderr)

    mod.set_axon_ntff_profile_hook(_hook)

BF16 = mybir.dt.bfloat16
F32 = mybir.dt.float32
NPBF = ml_dtypes.bfloat16

B, C, H, D = 4, 512, 8, 64
HPC = 4              # heads per core
CL = HPC * D         # 256 local channels
NCORES = 8
THETA = 10000.0
QC = 512             # q-chunk width (free dim per matmul)
KT = 128             # kt tile (partition dim of S^T tiles)
GROUP = 3            # kt-tiles per exp group (3 PSUM banks)
ACT_EXP = mybir.ActivationFunctionType.Exp


def build_nc(T: int) -> bass.Bass:
    PT = T // 128
    NJ = T // QC
    nc = bacc.Bacc()

    xT = nc.declare_dram_parameter("xT", [C, T], BF16, isOutput=False)
    wq = nc.declare_dram_parameter("wq", [C, CL], BF16, isOutput=False)
    wqs = nc.declare_dram_parameter("wqs", [C, CL], BF16, isOutput=False)
    wk = nc.declare_dram_parameter("wk", [C, CL], BF16, isOutput=False)
    wks = nc.declare_dram_parameter("wks", [C, CL], BF16, isOutput=False)
    wv = nc.declare_dram_parameter("wv", [C, CL], BF16, isOutput=False)
    wp = nc.declare_dram_parameter("wp", [CL, C], BF16, isOutput=False)
    cosb = nc.declare_dram_parameter("cosb", [CL, T], F32, isOutput=False)
    sinb = nc.declare_dram_parameter("sinb", [CL, T], F32, isOutput=False)
    msk = nc.declare_dram_parameter("msk", [4 * 128, QC], BF16, isOutput=False)
    y = nc.declare_dram_parameter("y", [T, C], F32, isOutput=True)

    with nc.allow_low_precision(
        reason="bf16 compute by design; f32 PSUM accumulation everywhere"
    ), tile.TileContext(nc) as tc, ExitStack() as ctx:
        pers = ctx.enter_context(tc.tile_pool(name="pers", bufs=1))
        work = ctx.enter_context(tc.tile_pool(name="work", bufs=4))
        pexp = ctx.enter_context(tc.tile_pool(name="pexp", bufs=3))
        big = ctx.enter_context(tc.tile_pool(name="big", bufs=2, space="PSUM"))
        b1 = ctx.enter_context(tc.tile_pool(name="b1", bufs=2, space="PSUM"))

        # ---------------- persistent SBUF: inputs ----------------
        xT_sb = [pers.tile([128, T], BF16, name=f"xT{i}", tag=f"xT{i}") for i in range(4)]
        for i in range(4):
            nc.sync.dma_start(out=xT_sb[i][:], in_=xT[128 * i:128 * i + 128, :])

        def load_w(handle, name):
            ts = [pers.tile([128, CL], BF16, name=f"{name}{i}", tag=f"{name}{i}") for i in range(4)]
            for i in range(4):
                nc.sync.dma_start(out=ts[i][:], in_=handle[128 * i:128 * i + 128, :])
            return ts

        wq_sb = load_w(wq, "wq")
        wqs_sb = load_w(wqs, "wqs")
        cos_sb = [pers.tile([128, T], F32, name=f"cos{i}", tag=f"cos{i}") for i in range(2)]
        sin_sb = [pers.tile([128, T], F32, name=f"sin{i}", tag=f"sin{i}") for i in range(2)]
        for i in range(2):
            nc.sync.dma_start(out=cos_sb[i][:], in_=cosb[128 * i:128 * i + 128, :])
            nc.sync.dma_start(out=sin_sb[i][:], in_=sinb[128 * i:128 * i + 128, :])
        wk_sb = load_w(wk, "wk")
        wks_sb = load_w(wks, "wks")
        wv_sb = load_w(wv, "wv")
        msk_sb = [pers.tile([128, QC], BF16, name=f"msk{r}", tag=f"msk{r}") for r in range(4)]
        for r in range(4):
            nc.sync.dma_start(out=msk_sb[r][:], in_=msk[128 * r:128 * r + 128, :])
        wp_sb = [pers.tile([128, C], BF16, name=f"wp{i}", tag=f"wp{i}") for i in range(2)]
        for i in range(2):
            nc.sync.dma_start(out=wp_sb[i][:], in_=wp[128 * i:128 * i + 128, :])

        # ---------------- persistent SBUF: intermediates ----------------
        qT_sb = [pers.tile([128, T], BF16, name=f"qT{i}", tag=f"qT{i}") for i in range(2)]
        kT_sb = [pers.tile([128, T], BF16, name=f"kT{i}", tag=f"kT{i}") for i in range(2)]
        vx_sb = [pers.tile([128, HPC * (D + 1)], BF16, name=f"vx{i}", tag=f"vx{i}") for i in range(PT)]
        rawT_sb = [pers.tile([128, T], F32, name=f"raw{i}", tag=f"raw{i}") for i in range(2)]
        rnT_sb = [pers.tile([128, T], BF16, name=f"rn{i}", tag=f"rn{i}") for i in range(2)]
        den_sb = pers.tile([1, HPC * T], BF16, name="den", tag="den")
        ones_sb = pers.tile([1, 64], BF16, name="ones", tag="ones")
        nc.vector.memset(ones_sb[:], 1.0)

        # ---------------- stage B: q/k projections + RoPE ----------------
        for wn, ws, dst in ((wq_sb, wqs_sb, qT_sb), (wk_sb, wks_sb, kT_sb)):
            for m in range(2):
                for t4 in range(NJ):
                    pq = big.tile([128, GROUP * 512], F32, name="big", tag="big")
                    tsl = slice(QC * t4, QC * t4 + QC)
                    for kc in range(4):
                        nc.tensor.matmul(
                            pq[:, 0:512],
                            lhsT=wn[kc][:, 128 * m:128 * m + 128],
                            rhs=xT_sb[kc][:, tsl],
                            start=(kc == 0),
                            stop=(kc == 3),
                        )
                    for kc in range(4):
                        nc.tensor.matmul(
                            pq[:, 512:1024],
                            lhsT=ws[kc][:, 128 * m:128 * m + 128],
                            rhs=xT_sb[kc][:, tsl],
                            start=(kc == 0),
                            stop=(kc == 3),
                        )
                    t1 = work.tile([128, 512], F32, name="t1", tag="t1")
                    t2 = work.tile([128, 512], F32, name="t2", tag="t2")
                    nc.vector.tensor_mul(t1[:], pq[:, 0:512], cos_sb[m][:, tsl])
                    nc.vector.tensor_mul(t2[:], pq[:, 512:1024], sin_sb[m][:, tsl])
                    nc.vector.tensor_add(dst[m][:, tsl], t1[:], t2[:])

        # ---------------- stage B2: v projection (natural layout + ones col) --
        for tt in range(PT):
            pv = b1.tile([128, 512], F32, name="b1", tag="b1")
            for kc in range(4):
                nc.tensor.matmul(
                    pv[:, 0:CL],
                    lhsT=xT_sb[kc][:, 128 * tt:128 * tt + 128],
                    rhs=wv_sb[kc][:],
                    start=(kc == 0),
                    stop=(kc == 3),
                )
            for h in range(HPC):
                nc.vector.tensor_copy(
                    vx_sb[tt][:, 65 * h:65 * h + 64], pv[:, 64 * h:64 * h + 64]
                )
                nc.vector.memset(vx_sb[tt][:, 65 * h + 64:65 * h + 65], 1.0)

        # ---------------- stage C: attention ----------------
        for h in range(HPC):
            ph, po = h // 2, 64 * (h % 2)
            for j in range(NJ):
                nkt = 4 * (j + 1)
                pvp = b1.tile([128, 512], F32, name="b1", tag="b1")
                qsl = slice(QC * j, QC * j + QC)
                its = list(range(nkt))
                for u0 in range(0, nkt, GROUP):
                    grp = its[u0:u0 + GROUP]
                    sg = big.tile([128, GROUP * 512], F32, name="big", tag="big")
                    for ui, it in enumerate(grp):
                        nc.tensor.matmul(
                            sg[:, 512 * ui:512 * ui + 512],
                            lhsT=kT_sb[ph][po:po + 64, 128 * it:128 * it + 128],
                            rhs=qT_sb[ph][po:po + 64, qsl],
                            start=True,
                            stop=True,
                        )
                    w = 512 * len(grp)
                    pg = pexp.tile([128, GROUP * 512], BF16, name="pg", tag="pg")
                    nc.scalar.activation(pg[:, 0:w], sg[:, 0:w], ACT_EXP, scale=0.125)
                    for ui, it in enumerate(grp):
                        psl = slice(512 * ui, 512 * ui + 512)
                        r = it - 4 * j
                        if r >= 0:
                            nc.vector.tensor_mul(pg[:, psl], pg[:, psl], msk_sb[r][:])
                        nc.tensor.matmul(
                            pvp[0:65, :],
                            lhsT=vx_sb[it][:, 65 * h:65 * h + 65],
                            rhs=pg[:, psl],
                            start=(it == 0),
                            stop=(it == nkt - 1),
                        )
                nc.vector.tensor_copy(rawT_sb[ph][po:po + 64, qsl], pvp[0:64, :])
                nc.vector.tensor_copy(
                    den_sb[0:1, h * T + QC * j:h * T + QC * j + QC], pvp[64:65, :]
                )

        # ---------------- stage N: normalize ----------------
        for ph in range(2):
            for j in range(NJ):
                qsl = slice(QC * j, QC * j + QC)
                bc = b1.tile([128, 512], F32, name="b1", tag="b1")
                for sub in range(2):
                    h = 2 * ph + sub
                    nc.tensor.matmul(
                        bc[64 * sub:64 * sub + 64, :],
                        lhsT=ones_sb[0:1, :],
                        rhs=den_sb[0:1, h * T + QC * j:h * T + QC * j + QC],
                        start=True,
                        stop=True,
                        tile_position=(0, 64 * sub),
                    )
                rbc = work.tile([128, 512], F32, name="rbc", tag="rbc")
                nc.vector.reciprocal_approx_fast(rbc[:], bc[:])
                nc.vector.tensor_mul(
                    rnT_sb[ph][:, qsl], rawT_sb[ph][:, qsl], rbc[:]
                )

        # ---------------- stage D: output projection ----------------
        for tt in range(PT):
            pp = b1.tile([128, 512], F32, name="b1", tag="b1")
            for kc in range(2):
                nc.tensor.matmul(
                    pp[:],
                    lhsT=rnT_sb[kc][:, 128 * tt:128 * tt + 128],
                    rhs=wp_sb[kc][:],
                    start=(kc == 0),
                    stop=(kc == 1),
                )
            ys = work.tile([128, 512], F32, name="ys", tag="ys")
            nc.vector.tensor_copy(ys[:], pp[:])
            nc.sync.dma_start(out=y[128 * tt:128 * tt + 128, :], in_=ys[:])

    nc.finalize()
    return nc


def prep_core_inputs(x, Wq, Wk, Wv, Wp, core, T):
    b, g = core // 2, core % 2
    sl = slice(CL * g, CL * g + CL)
    sw = np.arange(CL) ^ 1
    wq_ = np.ascontiguousarray(Wq[sl, :].T)
    wk_ = np.ascontiguousarray(Wk[sl, :].T)
    lc = np.arange(CL)
    gpair = (CL * g + lc) // 2
    invf = THETA ** (-(2.0 * gpair) / C)
    ang = np.arange(T)[None, :] * invf[:, None]
    cosb = np.cos(ang).astype(np.float32)
    sgn = np.where(lc % 2 == 0, -1.0, 1.0)
    sinb = (np.sin(ang) * sgn[:, None]).astype(np.float32)
    # causal 0/1 masks for the 4 diagonal kt-tiles of each q-chunk
    p = np.arange(128)[:, None]
    q = np.arange(QC)[None, :]
    m = np.concatenate(
        [(q >= 128 * r + p).astype(np.float32) for r in range(4)], axis=0
    )
    return {
        "xT": np.ascontiguousarray(x[b].T).astype(NPBF),
        "wq": wq_.astype(NPBF),
        "wqs": np.ascontiguousarray(wq_[:, sw]).astype(NPBF),
        "wk": wk_.astype(NPBF),
        "wks": np.ascontiguousarray(wk_[:, sw]).astype(NPBF),
        "wv": np.ascontiguousarray(Wv[sl, :].T).astype(NPBF),
        "wp": np.ascontiguousarray(Wp[:, sl].T).astype(NPBF),
        "cosb": cosb,
        "sinb": sinb,
        "msk": m.astype(NPBF),
    }


_NC_CACHE = {}


def _get_nc(T):
    if T not in _NC_CACHE:
        _NC_CACHE[T] = build_nc(T)
    return _NC_CACHE[T]


def kernel(x, Wq, Wk, Wv, Wp, bp, _trace=False):
    x = np.asarray(x, dtype=np.float32)
    Wq = np.asarray(Wq, dtype=np.float32)
    Wk = np.asarray(Wk, dtype=np.float32)
    Wv = np.asarray(Wv, dtype=np.float32)
    Wp = np.asarray(Wp, dtype=np.float32)
    bp = np.asarray(bp, dtype=np.float32)
    T = x.shape[1]
    nc = _get_nc(T)
    in_maps = [prep_core_inputs(x, Wq, Wk, Wv, Wp, c, T) for c in range(NCORES)]
    if _trace:
        _ensure_ntff_hook()
    res = run_bass_kernel_spmd(nc, in_maps, list(range(NCORES)), trace=_trace)
    out = np.zeros((B, T, C), np.float32)
    for b in range(B):
        out[b] = res.results[2 * b]["y"] + res.results[2 * b + 1]["y"]
    out += bp[None, None, :]
    if _trace:
        return out, res
    return out
